# revision 15
# baseline (speedup 1.0000x reference)
"""Trainium2 Bass kernel for nn_AttentionHead_6786048328376.

8-head spatial attention block: q/k/v 1x1-conv projections with additive
positional embedding on q/k, softmax over the QUERY axis (dim=2), attention
apply, channel-major output, 2-layer 1x1-conv MLP with mish, residual add.

Sharding: pure data-parallel over batch — 8 batch elements, one per
NeuronCore. Weights are replicated; no collectives.

Per-core design (C=512, N=H*W=1024, 8 heads, dh=32, ch=64):
  - x is held channel-major [512, 1024]; q/k land head-stacked [256, 1024]
    (row = 32*head + d) so head-pairs sit on 32-row PE array strips ->
    scores use 2-way row-tiled K=32 matmuls (tile_position).
  - scores are computed TRANSPOSED: sT[m, n] (key-major) so the softmax
    reduction over the query axis n is a free-axis reduction. Both n-halves
    of one head land in one 2-bank psum tile, so exp is a single [128,1024]
    scalar-engine pass per (head, m-chunk) writing bf16 (no max subtraction
    needed: |scores| <= ~8 at this problem's scale).
  - row-sums of exp are split between the scalar engine (fused accum_out)
    and the vector engine (tensor_reduce over the bf16 exp tile) to balance
    the two engines; the scalar engine is the kernel bottleneck.
  - v is computed directly transposed vT[n, c] = x.T @ Wv.T; the softmax
    1/sum is folded into vT rows (64 els/row) instead of dividing the 1M-el
    score matrix.
  - attention apply uses 2-way col-tiled bf16 matmuls (both heads of the
    pair concurrent, M=64) accumulating over m-chunks, producing attn
    channel-major [512, 1024] with no transposes.
  - all fp32 matmuls use float32r (fp32 lowers to 2 half-rate PE passes;
    f32r streams 1 row/cycle). The BIR verifier requires f32r operands to
    be produced as f32r, so those tiles/DRAM tensors are typed float32r.
  - mish(x) = x*tanh(ln(1+exp(x))) via Exp -> Ln(bias=1) -> Tanh on the
    scalar engine (phased to avoid activation-table thrash) plus vector ops.
"""

import numpy as np

_CACHE = {}

# of the 8 m-chunks per (head, pair-group), how many use ACT accum_out for
# the exp row-sum; the rest use a DVE tensor_reduce over the bf16 exp tile.
ACT_ACCUM_PER_8 = 3


def _build():
    import concourse.bacc as bacc
    import concourse.tile as tile
    import concourse.mybir as mybir

    dt = mybir.dt
    F32 = dt.float32
    F32R = dt.float32r
    BF16 = dt.bfloat16
    Act = mybir.ActivationFunctionType
    Alu = mybir.AluOpType
    AxX = mybir.AxisListType.X

    nc = bacc.Bacc("TRN2", target_bir_lowering=False, debug=False)

    x_d = nc.dram_tensor("x", [512, 1024], F32, kind="ExternalInput").ap()
    xbf_d = nc.dram_tensor("xbf", [512, 1024], BF16, kind="ExternalInput").ap()
    wqkt_d = nc.dram_tensor("wqkt", [512, 512], BF16, kind="ExternalInput").ap()
    peb_d = nc.dram_tensor("peb", [4, 128, 1024], BF16, kind="ExternalInput").ap()
    wvt_d = nc.dram_tensor("wvt", [512, 512], BF16, kind="ExternalInput").ap()
    bvb_d = nc.dram_tensor("bvb", [128, 512], F32, kind="ExternalInput").ap()
    w1t_d = nc.dram_tensor("w1t", [512, 512], F32R, kind="ExternalInput").ap()
    w2t_d = nc.dram_tensor("w2t", [512, 512], F32R, kind="ExternalInput").ap()
    b1_d = nc.dram_tensor("b1", [512, 1], F32, kind="ExternalInput").ap()
    b2_d = nc.dram_tensor("b2", [512, 1], F32, kind="ExternalInput").ap()
    out_d = nc.dram_tensor("out", [512, 1024], F32, kind="ExternalOutput").ap()

    with tile.TileContext(nc) as tc:
        with tc.tile_pool(name="persist", bufs=1) as per, \
             tc.tile_pool(name="mtmp", bufs=18) as mt, \
             tc.tile_pool(name="etp", bufs=12) as etp, \
             tc.tile_pool(name="small", bufs=20) as sm, \
             tc.tile_pool(name="sbig", bufs=3, space="PSUM") as ps, \
             tc.tile_pool(name="av", bufs=2, space="PSUM") as av:

            def ptile(shape, dtype, name):
                return per.tile(shape, dtype, name=name, tag=name)

            x_sb = [ptile([128, 1024], F32, f"xsb{i}") for i in range(4)]
            xbf_sb = [ptile([128, 1024], BF16, f"xbf{i}") for i in range(4)]
            wqk_sb = [ptile([128, 512], BF16, f"wqk{i}") for i in range(4)]
            pe_sb = [ptile([128, 1024], BF16, f"pe{i}") for i in range(4)]
            wv_sb = [ptile([128, 512], BF16, f"wv{i}") for i in range(4)]
            bv_sb = ptile([128, 512], F32, "bvsb")
            w1_sb = [ptile([128, 512], F32R, f"w1{i}") for i in range(4)]
            w2_sb = [ptile([128, 512], F32R, f"w2{i}") for i in range(4)]
            b1_sb = [ptile([128, 1], F32, f"b1c{i}") for i in range(4)]
            b2_sb = [ptile([128, 1], F32, f"b2c{i}") for i in range(4)]
            qk_sb = [ptile([128, 1024], BF16, f"qks{i}") for i in range(4)]
            vt_sb = [ptile([128, 512], F32, f"vts{i}") for i in range(8)]
            attn_sb = [ptile([128, 1024], F32R, f"attn{i}") for i in range(4)]
            mish_sb = [ptile([128, 1024], F32R, f"mish{i}") for i in range(4)]
            out_sb = [ptile([128, 1024], F32, f"osb{i}") for i in range(4)]

            dma = nc.sync.dma_start
            # issue the start-gating loads from different engines so their
            # DGE queues run in parallel (sync alone serializes ~0.7us each)
            for i in range(4):
                nc.sync.dma_start(out=xbf_sb[i], in_=xbf_d[128 * i:128 * (i + 1), :])
            for i in range(4):
                nc.scalar.dma_start(out=wqk_sb[i], in_=wqkt_d[128 * i:128 * (i + 1), :])
            for i in range(4):
                nc.gpsimd.dma_start(out=pe_sb[i], in_=peb_d[i])
            for i in range(4):
                nc.scalar.dma_start(out=wv_sb[i], in_=wvt_d[128 * i:128 * (i + 1), :])
            nc.gpsimd.dma_start(out=bv_sb, in_=bvb_d)

            mm = nc.tensor.matmul

            # q/k projections: qk[512, 1024] = WqkT.T @ x, then + (PE, bias)
            # pair-0's q/k rows (64-row halves of tiles 0 and 2) first so the
            # first scores matmuls start as early as possible; M=64 costs the
            # same stream cycles as M=128
            def proj_qk(t, r0, rn):
                for nh in range(2):
                    pt = ps.tile([128, 512], F32, name="pps", tag="sbig")
                    for kc in range(4):
                        mm(pt[0:rn, :],
                           lhsT=wqk_sb[kc][:, 128 * t + r0:128 * t + r0 + rn],
                           rhs=xbf_sb[kc][:, 512 * nh:512 * (nh + 1)],
                           start=(kc == 0), stop=(kc == 3))
                    nc.vector.tensor_add(
                        qk_sb[t][r0:r0 + rn, 512 * nh:512 * (nh + 1)],
                        pt[0:rn, :],
                        pe_sb[t][r0:r0 + rn, 512 * nh:512 * (nh + 1)])
            proj_qk(0, 0, 64)
            proj_qk(2, 0, 64)
            proj_qk(0, 64, 64)
            proj_qk(2, 64, 64)
            proj_qk(1, 0, 128)
            proj_qk(3, 0, 128)

            def project_vt(i):
                # vT[n, c] = x.T @ WvT, then + bv — emitted just-in-time
                # inside the first pair-group so exp work starts early
                pt = ps.tile([128, 512], F32, name="pps", tag="sbig")
                for kc in range(4):
                    mm(pt, lhsT=xbf_sb[kc][:, 128 * i:128 * (i + 1)],
                       rhs=wv_sb[kc],
                       start=(kc == 0), stop=(kc == 3))
                nc.vector.tensor_add(vt_sb[i], pt, bv_sb)

            # attention: four head-pair groups, each in two phases:
            # (1) scores + exp + row-sums + vts for all 8 m-chunks,
            # (2) a dense PE-only AV accumulation pass over the m-chunks.
            # Phase 2 of group pg overlaps phase 1 of group pg+1, keeping
            # the PE in long bursts instead of a per-m-chunk serial chain.
            for pg in range(4):
                g = pg // 2           # which 128-row q/k tile
                off0 = 64 * (pg % 2)  # partition offset of this pair in it
                q_t = qk_sb[g]
                k_t = qk_sb[2 + g]
                ets, vtss = {}, {}
                interleave_av = True
                if interleave_av:
                    avt = [av.tile([128, 512], F32, name="avt", tag="av")
                           for _ in range(2)]  # [nh]
                for mc in range(8):
                    if pg == 0:
                        project_vt(mc)
                    S = sm.tile([128, 2], F32, name="S", tag="S")
                    R = sm.tile([128, 2], F32, name="R", tag="R")
                    for hp in range(2):
                        off = off0 + 32 * hp
                        sp = ps.tile([128, 1024], F32, name="sps", tag="sbig")
                        for nh in range(2):
                            mm(sp[:, 512 * nh:512 * (nh + 1)],
                               lhsT=k_t[off:off + 32, 128 * mc:128 * (mc + 1)],
                               rhs=q_t[off:off + 32, 512 * nh:512 * (nh + 1)],
                               start=True, stop=True,
                               tile_position=(off, 0))
                        et = etp.tile([128, 1024], BF16, name="et", tag="et")
                        if mc % 8 < ACT_ACCUM_PER_8:
                            nc.scalar.activation(et, sp, Act.Exp,
                                                 accum_out=S[:, hp:hp + 1])
                        else:
                            nc.scalar.activation(et, sp, Act.Exp)
                            nc.vector.tensor_reduce(
                                S[:, hp:hp + 1], et, axis=AxX, op=Alu.add)
                        ets[(mc, hp)] = et
                    nc.vector.reciprocal(R, S)
                    for hp in range(2):
                        h = 2 * pg + hp
                        vts = sm.tile([128, 64], BF16, name="vtsc", tag="vtsc")
                        nc.vector.tensor_scalar_mul(
                            vts, vt_sb[mc][:, 64 * h:64 * (h + 1)],
                            R[:, hp:hp + 1])
                        vtss[(mc, hp)] = vts
                        if interleave_av:
                            for nh in range(2):
                                mm(avt[nh][64 * hp:64 * hp + 64, :],
                                   lhsT=vts,
                                   rhs=ets[(mc, hp)][:, 512 * nh:512 * (nh + 1)],
                                   start=(mc == 0), stop=(mc == 7),
                                   tile_position=(0, 64 * hp),
                                   skip_group_check=True)
                if not interleave_av:
                    avt = [av.tile([128, 512], F32, name="avt", tag="av")
                           for _ in range(2)]  # [nh]
                for mc in range(8 if not interleave_av else 0):
                    for hp in range(2):
                        for nh in range(2):
                            # two col-tiled accumulation series share each
                            # bank on disjoint partition halves; has_written
                            # is per-element so this is safe — the sim's
                            # coarse zero-region tracker is what we skip.
                            mm(avt[nh][64 * hp:64 * hp + 64, :],
                               lhsT=vtss[(mc, hp)],
                               rhs=ets[(mc, hp)][:, 512 * nh:512 * (nh + 1)],
                               start=(mc == 0), stop=(mc == 7),
                               tile_position=(0, 64 * hp),
                               skip_group_check=True)
                for nh in range(2):
                    nc.vector.tensor_copy(
                        out=attn_sb[pg][:, 512 * nh:512 * (nh + 1)],
                        in_=avt[nh])

            # MLP weight/bias loads deferred here so the early DMA queues
            # serve the tensors that gate the first scores matmuls
            for i in range(4):
                dma(out=x_sb[i], in_=x_d[128 * i:128 * (i + 1), :])
            for i in range(4):
                dma(out=w1_sb[i], in_=w1t_d[128 * i:128 * (i + 1), :])
                dma(out=w2_sb[i], in_=w2t_d[128 * i:128 * (i + 1), :])
                dma(out=b1_sb[i], in_=b1_d[128 * i:128 * (i + 1), :])
                dma(out=b2_sb[i], in_=b2_d[128 * i:128 * (i + 1), :])

            # MLP: h1 = W1 @ attn + b1; mish; out = W2 @ mish + b2 + x
            # full per-nh chains: MLP2 of nh=0 overlaps nh=1's mish chain
            h1f, t_t, sp_t, th_t = {}, {}, {}, {}
            for nh in range(2):
                for i in range(4):
                    pt = ps.tile([128, 512], F32, name="h1ps", tag="sbig")
                    for kc in range(4):
                        mm(pt, lhsT=w1_sb[kc][:, 128 * i:128 * (i + 1)],
                           rhs=attn_sb[kc][:, 512 * nh:512 * (nh + 1)],
                           start=(kc == 0), stop=(kc == 3))
                    tt = mt.tile([128, 512], F32, name="mtt", tag="mtt")
                    nc.scalar.activation(tt, pt, Act.Exp, bias=b1_sb[i])
                    t_t[(nh, i)] = tt
                    hf = mt.tile([128, 512], F32, name="mtt", tag="mtt")
                    nc.vector.tensor_scalar_add(hf, pt, b1_sb[i])
                    h1f[(nh, i)] = hf
            for nh in range(2):
                for i in range(4):
                    spt = mt.tile([128, 512], F32, name="mtt", tag="mtt")
                    nc.scalar.activation(spt, t_t[(nh, i)], Act.Ln, bias=1.0)
                    sp_t[(nh, i)] = spt
            def mlp2(nh):
                for j in range(4):
                    pt = ps.tile([128, 512], F32, name="h2ps", tag="sbig")
                    for kc in range(4):
                        mm(pt, lhsT=w2_sb[kc][:, 128 * j:128 * (j + 1)],
                           rhs=mish_sb[kc][:, 512 * nh:512 * (nh + 1)],
                           start=(kc == 0), stop=(kc == 3))
                    nc.vector.scalar_tensor_tensor(
                        out=out_sb[j][:, 512 * nh:512 * (nh + 1)],
                        in0=pt, scalar=b2_sb[j],
                        in1=x_sb[j][:, 512 * nh:512 * (nh + 1)],
                        op0=Alu.add, op1=Alu.add)

            # tanh shares a table set with exp, so per-nh chains cost no
            # extra loads; MLP2 of nh=0 overlaps the nh=1 chain on ACT.
            for nh in range(2):
                for i in range(4):
                    tht = mt.tile([128, 512], F32, name="mtt", tag="mtt")
                    nc.scalar.activation(tht, sp_t[(nh, i)], Act.Tanh)
                    th_t[(nh, i)] = tht
                for i in range(4):
                    nc.vector.tensor_mul(
                        mish_sb[i][:, 512 * nh:512 * (nh + 1)],
                        h1f[(nh, i)], th_t[(nh, i)])
                mlp2(nh)
            for j in range(4):
                dma(out=out_d[128 * j:128 * (j + 1), :], in_=out_sb[j])

    nc.compile()
    return nc


def _get_nc():
    if "nc" not in _CACHE:
        _CACHE["nc"] = _build()
    return _CACHE["nc"]


def _make_in_maps(inputs):
    x = np.asarray(inputs["x"], np.float32)
    PE = np.asarray(inputs["PE"], np.float32)
    Wq = np.asarray(inputs["Wq"], np.float32)
    bq = np.asarray(inputs["bq"], np.float32)
    Wk = np.asarray(inputs["Wk"], np.float32)
    bk = np.asarray(inputs["bk"], np.float32)
    Wv = np.asarray(inputs["Wv"], np.float32)
    bv = np.asarray(inputs["bv"], np.float32)
    W1 = np.asarray(inputs["W1"], np.float32)
    b1 = np.asarray(inputs["b1"], np.float32)
    W2 = np.asarray(inputs["W2"], np.float32)
    b2 = np.asarray(inputs["b2"], np.float32)

    import ml_dtypes
    s = np.float32(1.0 / np.sqrt(np.float32(32.0)))
    pef = PE.reshape(32, 1024)
    pe4 = np.tile(pef, (4, 1))  # [128, 1024], row = 32*j + d
    peb = np.stack([
        s * (pe4 + bq[0:128][:, None]),
        s * (pe4 + bq[128:256][:, None]),
        pe4 + bk[0:128][:, None],
        pe4 + bk[128:256][:, None],
    ]).astype(ml_dtypes.bfloat16)
    wqkt = np.ascontiguousarray(
        np.concatenate([s * Wq, Wk], axis=0).T.astype(ml_dtypes.bfloat16))
    wvt = np.ascontiguousarray(Wv.T.astype(ml_dtypes.bfloat16))
    bvb = np.ascontiguousarray(
        np.broadcast_to(bv, (128, 512)).astype(np.float32))
    w1t = np.ascontiguousarray(W1.T.astype(np.float32))
    w2t = np.ascontiguousarray(W2.T.astype(np.float32))
    b1c = np.ascontiguousarray(b1.reshape(512, 1).astype(np.float32))
    b2c = np.ascontiguousarray(b2.reshape(512, 1).astype(np.float32))

    xb = np.ascontiguousarray(x.reshape(8, 512, 1024))
    xbf = xb.astype(ml_dtypes.bfloat16)
    shared = dict(wqkt=wqkt, peb=peb, wvt=wvt, bvb=bvb,
                  w1t=w1t, w2t=w2t, b1=b1c, b2=b2c)
    return [dict(x=np.ascontiguousarray(xb[i]),
                 xbf=np.ascontiguousarray(xbf[i]), **shared)
            for i in range(8)]


def _run(in_maps, trace=False, **kwargs):
    from concourse import bass_utils
    nc = _get_nc()
    return bass_utils.run_bass_kernel_spmd(
        nc, in_maps, core_ids=list(range(8)), trace=trace, **kwargs)


def kernel(**inputs):
    in_maps = _make_in_maps(inputs)
    res = _run(in_maps)
    out = np.stack([r["out"] for r in res.results], axis=0)
    return np.ascontiguousarray(out.reshape(8, 512, 32, 32).astype(np.float32))


# revision 16
# speedup vs baseline: 1.0304x; 1.0304x over previous
"""Trainium2 Bass kernel for nn_AttentionHead_6786048328376.

8-head spatial attention block: q/k/v 1x1-conv projections with additive
positional embedding on q/k, softmax over the QUERY axis (dim=2), attention
apply, channel-major output, 2-layer 1x1-conv MLP with mish, residual add.

Sharding: pure data-parallel over batch — 8 batch elements, one per
NeuronCore. Weights are replicated; no collectives.

Per-core design (C=512, N=H*W=1024, 8 heads, dh=32, ch=64):
  - x is held channel-major [512, 1024]; q/k land head-stacked [256, 1024]
    (row = 32*head + d) so head-pairs sit on 32-row PE array strips ->
    scores use 2-way row-tiled K=32 matmuls (tile_position).
  - scores are computed TRANSPOSED: sT[m, n] (key-major) so the softmax
    reduction over the query axis n is a free-axis reduction. Both n-halves
    of one head land in one 2-bank psum tile, so exp is a single [128,1024]
    scalar-engine pass per (head, m-chunk) writing bf16 (no max subtraction
    needed: |scores| <= ~8 at this problem's scale).
  - row-sums of exp are split between the scalar engine (fused accum_out)
    and the vector engine (tensor_reduce over the bf16 exp tile) to balance
    the two engines; the scalar engine is the kernel bottleneck.
  - v is computed directly transposed vT[n, c] = x.T @ Wv.T; the softmax
    1/sum is folded into vT rows (64 els/row) instead of dividing the 1M-el
    score matrix.
  - attention apply uses 2-way col-tiled bf16 matmuls (both heads of the
    pair concurrent, M=64) accumulating over m-chunks, producing attn
    channel-major [512, 1024] with no transposes.
  - all fp32 matmuls use float32r (fp32 lowers to 2 half-rate PE passes;
    f32r streams 1 row/cycle). The BIR verifier requires f32r operands to
    be produced as f32r, so those tiles/DRAM tensors are typed float32r.
  - mish(x) = x*tanh(ln(1+exp(x))) via Exp -> Ln(bias=1) -> Tanh on the
    scalar engine (phased to avoid activation-table thrash) plus vector ops.
"""

import numpy as np

_CACHE = {}

# of the 8 m-chunks per (head, pair-group), how many use ACT accum_out for
# the exp row-sum; the rest use a DVE tensor_reduce over the bf16 exp tile.
ACT_ACCUM_PER_8 = 4


def _build():
    import concourse.bacc as bacc
    import concourse.tile as tile
    import concourse.mybir as mybir

    dt = mybir.dt
    F32 = dt.float32
    F32R = dt.float32r
    BF16 = dt.bfloat16
    Act = mybir.ActivationFunctionType
    Alu = mybir.AluOpType
    AxX = mybir.AxisListType.X

    nc = bacc.Bacc("TRN2", target_bir_lowering=False, debug=False)

    x_d = nc.dram_tensor("x", [512, 1024], F32, kind="ExternalInput").ap()
    xbf_d = nc.dram_tensor("xbf", [512, 1024], BF16, kind="ExternalInput").ap()
    wqkt_d = nc.dram_tensor("wqkt", [512, 512], BF16, kind="ExternalInput").ap()
    peb_d = nc.dram_tensor("peb", [4, 128, 1024], BF16, kind="ExternalInput").ap()
    wvt_d = nc.dram_tensor("wvt", [512, 512], BF16, kind="ExternalInput").ap()
    bvb_d = nc.dram_tensor("bvb", [128, 512], F32, kind="ExternalInput").ap()
    w1t_d = nc.dram_tensor("w1t", [512, 512], F32R, kind="ExternalInput").ap()
    w2t_d = nc.dram_tensor("w2t", [512, 512], F32R, kind="ExternalInput").ap()
    b1_d = nc.dram_tensor("b1", [512, 1], F32, kind="ExternalInput").ap()
    b2_d = nc.dram_tensor("b2", [512, 1], F32, kind="ExternalInput").ap()
    out_d = nc.dram_tensor("out", [512, 1024], F32, kind="ExternalOutput").ap()

    with tile.TileContext(nc) as tc:
        with tc.tile_pool(name="persist", bufs=1) as per, \
             tc.tile_pool(name="mtmp", bufs=18) as mt, \
             tc.tile_pool(name="etp", bufs=16) as etp, \
             tc.tile_pool(name="small", bufs=20) as sm, \
             tc.tile_pool(name="sbig", bufs=3, space="PSUM") as ps, \
             tc.tile_pool(name="av", bufs=2, space="PSUM") as av:

            def ptile(shape, dtype, name):
                return per.tile(shape, dtype, name=name, tag=name)

            x_sb = [ptile([128, 1024], F32, f"xsb{i}") for i in range(4)]
            xbf_sb = [ptile([128, 1024], BF16, f"xbf{i}") for i in range(4)]
            wqk_sb = [ptile([128, 512], BF16, f"wqk{i}") for i in range(4)]
            pe_sb = [ptile([128, 1024], BF16, f"pe{i}") for i in range(4)]
            wv_sb = [ptile([128, 512], BF16, f"wv{i}") for i in range(4)]
            bv_sb = ptile([128, 512], F32, "bvsb")
            w1_sb = [ptile([128, 512], F32R, f"w1{i}") for i in range(4)]
            w2_sb = [ptile([128, 512], F32R, f"w2{i}") for i in range(4)]
            b1_sb = [ptile([128, 1], F32, f"b1c{i}") for i in range(4)]
            b2_sb = [ptile([128, 1], F32, f"b2c{i}") for i in range(4)]
            qk_sb = [ptile([128, 1024], BF16, f"qks{i}") for i in range(4)]
            vt_sb = [ptile([128, 512], F32, f"vts{i}") for i in range(8)]
            attn_sb = [ptile([128, 1024], F32R, f"attn{i}") for i in range(4)]
            mish_sb = [ptile([128, 1024], F32R, f"mish{i}") for i in range(4)]
            out_sb = [ptile([128, 1024], F32, f"osb{i}") for i in range(4)]

            dma = nc.sync.dma_start
            # issue the start-gating loads from different engines so their
            # DGE queues run in parallel (sync alone serializes ~0.7us each)
            for i in range(4):
                nc.sync.dma_start(out=xbf_sb[i], in_=xbf_d[128 * i:128 * (i + 1), :])
            for i in range(4):
                nc.scalar.dma_start(out=wqk_sb[i], in_=wqkt_d[128 * i:128 * (i + 1), :])
            for i in range(4):
                nc.gpsimd.dma_start(out=pe_sb[i], in_=peb_d[i])
            for i in range(4):
                nc.scalar.dma_start(out=wv_sb[i], in_=wvt_d[128 * i:128 * (i + 1), :])
            nc.gpsimd.dma_start(out=bv_sb, in_=bvb_d)

            mm = nc.tensor.matmul

            # q/k projections: qk[512, 1024] = WqkT.T @ x, then + (PE, bias)
            # pair-0's q/k rows (64-row halves of tiles 0 and 2) first so the
            # first scores matmuls start as early as possible; M=64 costs the
            # same stream cycles as M=128
            def proj_qk(t, r0, rn):
                for nh in range(2):
                    pt = ps.tile([128, 512], F32, name="pps", tag="sbig")
                    for kc in range(4):
                        mm(pt[0:rn, :],
                           lhsT=wqk_sb[kc][:, 128 * t + r0:128 * t + r0 + rn],
                           rhs=xbf_sb[kc][:, 512 * nh:512 * (nh + 1)],
                           start=(kc == 0), stop=(kc == 3))
                    nc.vector.tensor_add(
                        qk_sb[t][r0:r0 + rn, 512 * nh:512 * (nh + 1)],
                        pt[0:rn, :],
                        pe_sb[t][r0:r0 + rn, 512 * nh:512 * (nh + 1)])
            proj_qk(0, 0, 128)
            proj_qk(2, 0, 128)
            proj_qk(1, 0, 128)
            proj_qk(3, 0, 128)

            def project_vt(i):
                # vT[n, c] = x.T @ WvT, then + bv — emitted just-in-time
                # inside the first pair-group so exp work starts early
                pt = ps.tile([128, 512], F32, name="pps", tag="sbig")
                for kc in range(4):
                    mm(pt, lhsT=xbf_sb[kc][:, 128 * i:128 * (i + 1)],
                       rhs=wv_sb[kc],
                       start=(kc == 0), stop=(kc == 3))
                nc.vector.tensor_add(vt_sb[i], pt, bv_sb)

            # attention: four head-pair groups, each in two phases:
            # (1) scores + exp + row-sums + vts for all 8 m-chunks,
            # (2) a dense PE-only AV accumulation pass over the m-chunks.
            # Phase 2 of group pg overlaps phase 1 of group pg+1, keeping
            # the PE in long bursts instead of a per-m-chunk serial chain.
            for pg in range(4):
                g = pg // 2           # which 128-row q/k tile
                off0 = 64 * (pg % 2)  # partition offset of this pair in it
                q_t = qk_sb[g]
                k_t = qk_sb[2 + g]
                ets, vtss = {}, {}
                interleave_av = True
                if interleave_av:
                    avt = [av.tile([128, 512], F32, name="avt", tag="av")
                           for _ in range(2)]  # [nh]
                for mc in range(8):
                    if pg == 0:
                        project_vt(mc)
                    S = sm.tile([128, 2], F32, name="S", tag="S")
                    R = sm.tile([128, 2], F32, name="R", tag="R")
                    for hp in range(2):
                        off = off0 + 32 * hp
                        sp = ps.tile([128, 1024], F32, name="sps", tag="sbig")
                        for nh in range(2):
                            mm(sp[:, 512 * nh:512 * (nh + 1)],
                               lhsT=k_t[off:off + 32, 128 * mc:128 * (mc + 1)],
                               rhs=q_t[off:off + 32, 512 * nh:512 * (nh + 1)],
                               start=True, stop=True,
                               tile_position=(off, 0))
                        et = etp.tile([128, 1024], BF16, name="et", tag="et")
                        if mc % 8 < ACT_ACCUM_PER_8:
                            nc.scalar.activation(et, sp, Act.Exp,
                                                 accum_out=S[:, hp:hp + 1])
                        else:
                            nc.scalar.activation(et, sp, Act.Exp)
                            nc.vector.tensor_reduce(
                                S[:, hp:hp + 1], et, axis=AxX, op=Alu.add)
                        ets[(mc, hp)] = et
                    nc.vector.reciprocal(R, S)
                    for hp in range(2):
                        h = 2 * pg + hp
                        vts = sm.tile([128, 64], BF16, name="vtsc", tag="vtsc")
                        nc.vector.tensor_scalar_mul(
                            vts, vt_sb[mc][:, 64 * h:64 * (h + 1)],
                            R[:, hp:hp + 1])
                        vtss[(mc, hp)] = vts
                        if interleave_av:
                            for nh in range(2):
                                mm(avt[nh][64 * hp:64 * hp + 64, :],
                                   lhsT=vts,
                                   rhs=ets[(mc, hp)][:, 512 * nh:512 * (nh + 1)],
                                   start=(mc == 0), stop=(mc == 7),
                                   tile_position=(0, 64 * hp),
                                   skip_group_check=True)
                if not interleave_av:
                    avt = [av.tile([128, 512], F32, name="avt", tag="av")
                           for _ in range(2)]  # [nh]
                for mc in range(8 if not interleave_av else 0):
                    for hp in range(2):
                        for nh in range(2):
                            # two col-tiled accumulation series share each
                            # bank on disjoint partition halves; has_written
                            # is per-element so this is safe — the sim's
                            # coarse zero-region tracker is what we skip.
                            mm(avt[nh][64 * hp:64 * hp + 64, :],
                               lhsT=vtss[(mc, hp)],
                               rhs=ets[(mc, hp)][:, 512 * nh:512 * (nh + 1)],
                               start=(mc == 0), stop=(mc == 7),
                               tile_position=(0, 64 * hp),
                               skip_group_check=True)
                for nh in range(2):
                    nc.vector.tensor_copy(
                        out=attn_sb[pg][:, 512 * nh:512 * (nh + 1)],
                        in_=avt[nh])

            # MLP weight/bias loads deferred here so the early DMA queues
            # serve the tensors that gate the first scores matmuls
            for i in range(4):
                dma(out=x_sb[i], in_=x_d[128 * i:128 * (i + 1), :])
            for i in range(4):
                dma(out=w1_sb[i], in_=w1t_d[128 * i:128 * (i + 1), :])
                dma(out=w2_sb[i], in_=w2t_d[128 * i:128 * (i + 1), :])
                dma(out=b1_sb[i], in_=b1_d[128 * i:128 * (i + 1), :])
                dma(out=b2_sb[i], in_=b2_d[128 * i:128 * (i + 1), :])

            # MLP: h1 = W1 @ attn + b1; mish; out = W2 @ mish + b2 + x
            # full per-nh chains: MLP2 of nh=0 overlaps nh=1's mish chain
            h1f, t_t, sp_t, th_t = {}, {}, {}, {}
            for nh in range(2):
                for i in range(4):
                    pt = ps.tile([128, 512], F32, name="h1ps", tag="sbig")
                    for kc in range(4):
                        mm(pt, lhsT=w1_sb[kc][:, 128 * i:128 * (i + 1)],
                           rhs=attn_sb[kc][:, 512 * nh:512 * (nh + 1)],
                           start=(kc == 0), stop=(kc == 3))
                    tt = mt.tile([128, 512], F32, name="mtt", tag="mtt")
                    nc.scalar.activation(tt, pt, Act.Exp, bias=b1_sb[i])
                    t_t[(nh, i)] = tt
                    hf = mt.tile([128, 512], F32, name="mtt", tag="mtt")
                    nc.vector.tensor_scalar_add(hf, pt, b1_sb[i])
                    h1f[(nh, i)] = hf
            for nh in range(2):
                for i in range(4):
                    spt = mt.tile([128, 512], F32, name="mtt", tag="mtt")
                    nc.scalar.activation(spt, t_t[(nh, i)], Act.Ln, bias=1.0)
                    sp_t[(nh, i)] = spt
            def mlp2(nh):
                for j in range(4):
                    pt = ps.tile([128, 512], F32, name="h2ps", tag="sbig")
                    for kc in range(4):
                        mm(pt, lhsT=w2_sb[kc][:, 128 * j:128 * (j + 1)],
                           rhs=mish_sb[kc][:, 512 * nh:512 * (nh + 1)],
                           start=(kc == 0), stop=(kc == 3))
                    nc.vector.scalar_tensor_tensor(
                        out=out_sb[j][:, 512 * nh:512 * (nh + 1)],
                        in0=pt, scalar=b2_sb[j],
                        in1=x_sb[j][:, 512 * nh:512 * (nh + 1)],
                        op0=Alu.add, op1=Alu.add)

            # tanh shares a table set with exp, so per-nh chains cost no
            # extra loads; MLP2 of nh=0 overlaps the nh=1 chain on ACT.
            for nh in range(2):
                for i in range(4):
                    tht = mt.tile([128, 512], F32, name="mtt", tag="mtt")
                    nc.scalar.activation(tht, sp_t[(nh, i)], Act.Tanh)
                    th_t[(nh, i)] = tht
                for i in range(4):
                    nc.vector.tensor_mul(
                        mish_sb[i][:, 512 * nh:512 * (nh + 1)],
                        h1f[(nh, i)], th_t[(nh, i)])
                mlp2(nh)
            for j in range(4):
                dma(out=out_d[128 * j:128 * (j + 1), :], in_=out_sb[j])

    nc.compile()
    return nc


def _get_nc():
    if "nc" not in _CACHE:
        _CACHE["nc"] = _build()
    return _CACHE["nc"]


def _make_in_maps(inputs):
    x = np.asarray(inputs["x"], np.float32)
    PE = np.asarray(inputs["PE"], np.float32)
    Wq = np.asarray(inputs["Wq"], np.float32)
    bq = np.asarray(inputs["bq"], np.float32)
    Wk = np.asarray(inputs["Wk"], np.float32)
    bk = np.asarray(inputs["bk"], np.float32)
    Wv = np.asarray(inputs["Wv"], np.float32)
    bv = np.asarray(inputs["bv"], np.float32)
    W1 = np.asarray(inputs["W1"], np.float32)
    b1 = np.asarray(inputs["b1"], np.float32)
    W2 = np.asarray(inputs["W2"], np.float32)
    b2 = np.asarray(inputs["b2"], np.float32)

    import ml_dtypes
    s = np.float32(1.0 / np.sqrt(np.float32(32.0)))
    pef = PE.reshape(32, 1024)
    pe4 = np.tile(pef, (4, 1))  # [128, 1024], row = 32*j + d
    peb = np.stack([
        s * (pe4 + bq[0:128][:, None]),
        s * (pe4 + bq[128:256][:, None]),
        pe4 + bk[0:128][:, None],
        pe4 + bk[128:256][:, None],
    ]).astype(ml_dtypes.bfloat16)
    wqkt = np.ascontiguousarray(
        np.concatenate([s * Wq, Wk], axis=0).T.astype(ml_dtypes.bfloat16))
    wvt = np.ascontiguousarray(Wv.T.astype(ml_dtypes.bfloat16))
    bvb = np.ascontiguousarray(
        np.broadcast_to(bv, (128, 512)).astype(np.float32))
    w1t = np.ascontiguousarray(W1.T.astype(np.float32))
    w2t = np.ascontiguousarray(W2.T.astype(np.float32))
    b1c = np.ascontiguousarray(b1.reshape(512, 1).astype(np.float32))
    b2c = np.ascontiguousarray(b2.reshape(512, 1).astype(np.float32))

    xb = np.ascontiguousarray(x.reshape(8, 512, 1024))
    xbf = xb.astype(ml_dtypes.bfloat16)
    shared = dict(wqkt=wqkt, peb=peb, wvt=wvt, bvb=bvb,
                  w1t=w1t, w2t=w2t, b1=b1c, b2=b2c)
    return [dict(x=np.ascontiguousarray(xb[i]),
                 xbf=np.ascontiguousarray(xbf[i]), **shared)
            for i in range(8)]


def _run(in_maps, trace=False, **kwargs):
    from concourse import bass_utils
    nc = _get_nc()
    return bass_utils.run_bass_kernel_spmd(
        nc, in_maps, core_ids=list(range(8)), trace=trace, **kwargs)


def kernel(**inputs):
    in_maps = _make_in_maps(inputs)
    res = _run(in_maps)
    out = np.stack([r["out"] for r in res.results], axis=0)
    return np.ascontiguousarray(out.reshape(8, 512, 32, 32).astype(np.float32))
